# revision 1
# baseline (speedup 1.0000x reference)
"""Trainium2 Bass kernel for a 2-layer complex-gated GRU.

Problem (hardcoded):
  B=128, T=128, IN=256, H=2048, OUT=64, fp32.
  reference: 2 stacked complex GRU cells scanned over T, then a complex FC.

Strategy: 8-way tensor-parallel over the gate output dim (each core owns a
128-wide slice of the 1024 complex gate outputs => 256 rows of the 2048-row
real-valued hidden state). Everything lives transposed ([features, batch]) so
pre-transposed weights are the stationary matmul operand and activations are
the moving operand; matmul outputs land already transposed for the next step.
Per step, each cell needs two 8-core all-gathers (r gate, then h'). The two
cells are software-pipelined (cell0 at t runs with cell1 at t-1) so collective
latency hides under the other cell's matmuls. Matmul operands are bf16 (4x
faster PE), accumulation is fp32 in PSUM, and the carried state update
h' = (1-z)h + z*h~ is computed in fp32 from a core-local fp32 shard.
"""

import functools
import os

import numpy as np
import ml_dtypes

B, T, IN, H, OUT = 128, 128, 256, 2048, 64
NCORES = 8
M = H // 2  # 1024 complex gate outputs per gate
SH = M // NCORES  # 128 complex outputs per core per gate
K0 = IN + H  # 2304 = layer-0 contraction length (real-valued)
K1 = H + H  # 4096 = layer-1 contraction length
NK0 = K0 // 128  # 18
NK1 = K1 // 128  # 32
NHT = H // 128  # 16 tiles of the hidden state

BF16 = ml_dtypes.bfloat16


def _h_perm():
    """h storage layout: row p holds, for shard s=p//256, q=p%256:
    q<128 -> real component s*128+q ; q>=128 -> imag component s*128+(q-128).
    Returns perm such that h_layout[p] = h_natural[perm[p]] with
    h_natural = [hr (1024), hi (1024)]."""
    p = np.arange(H)
    s = p // 256
    q = p % 256
    comp = s * 128 + (q % 128)
    return np.where(q < 128, comp, M + comp)


H_PERM = _h_perm()


def _col_perm_layer0():
    """Contraction row t -> column of A=[cr-block | ci-block] (width 2*n0).
    Row layout: [xr (128), xi (128), h in H_PERM layout (2048)].
    cr = [xr(128), hr(1024)], ci = [xi(128), hi(1024)]."""
    n0 = K0 // 2  # 1152
    ih = IN // 2  # 128
    cols = np.empty(K0, np.int64)
    t = np.arange(K0)
    # x real rows
    cols[:ih] = t[:ih]
    # x imag rows
    cols[ih : 2 * ih] = n0 + (t[ih : 2 * ih] - ih)
    # h rows (perm layout)
    p = t[2 * ih :] - 2 * ih
    comp = H_PERM[p]  # natural component index in [0, 2048)
    is_real = comp < M
    c = np.where(is_real, comp, comp - M)
    cols[2 * ih :] = np.where(is_real, ih + c, n0 + ih + c)
    return cols


def _col_perm_layer1():
    """Row layout: [h0 in H_PERM layout (2048), h1 in H_PERM layout (2048)].
    'x' of cell1 is h0. cr = [h0r(1024), h1r(1024)], ci = [h0i, h1i]."""
    n1 = K1 // 2  # 2048
    ih = H // 2  # 1024
    cols = np.empty(K1, np.int64)
    for blk in range(2):  # 0: h0 (x-part), 1: h1 (h-part)
        p = np.arange(H)
        comp = H_PERM[p]
        is_real = comp < M
        c = np.where(is_real, comp, comp - M)
        base_r = blk * ih  # position inside cr
        base_i = n1 + blk * ih  # position inside ci
        cols[blk * H : (blk + 1) * H] = np.where(is_real, base_r + c, base_i + c)
    return cols


COL_PERM0 = _col_perm_layer0()
COL_PERM1 = _col_perm_layer1()


def _sbuf_layout(w, ncols):
    """[K, ncols] -> [128, (K//128)*ncols] with block k at cols [k*ncols:(k+1)*ncols]."""
    K = w.shape[0]
    nk = K // 128
    return np.ascontiguousarray(
        w.reshape(nk, 128, ncols).transpose(1, 0, 2).reshape(128, nk * ncols)
    )


def _gate_weights(Wr, Wi, col_perm, s):
    """Per-core stationary weights for one layer.

    Wr/Wi: [3, M, n] (gates z, r, h~). Returns (wzr [K,512], wh [K,256]) where
    output col blocks are [z_r, z_i, r_r, r_i] and [h_r, h_i], each 128 wide
    (core s's slice), and rows follow col_perm's contraction layout."""
    sl = slice(s * SH, (s + 1) * SH)
    blocks_zr = []
    for g in range(2):
        A_r = np.hstack([Wr[g], -Wi[g]])  # real output rows
        A_i = np.hstack([Wi[g], Wr[g]])  # imag output rows
        blocks_zr.append(A_r[sl][:, col_perm].T)
        blocks_zr.append(A_i[sl][:, col_perm].T)
    wzr = np.concatenate(blocks_zr, axis=1)
    A_r = np.hstack([Wr[2], -Wi[2]])
    A_i = np.hstack([Wi[2], Wr[2]])
    wh = np.concatenate([A_r[sl][:, col_perm].T, A_i[sl][:, col_perm].T], axis=1)
    return wzr, wh


def _gate_biases(br, bi, s):
    """[6 cols] = z_r, z_i, r_r, r_i, h_r, h_i for core s; each col 128 long."""
    sl = slice(s * SH, (s + 1) * SH)
    return np.stack(
        [br[0][sl], bi[0][sl], br[1][sl], bi[1][sl], br[2][sl], bi[2][sl]], axis=1
    )


def host_prep(inputs):
    """Build per-core in_maps (numpy) from the full problem inputs."""
    x = np.asarray(inputs["x"], np.float32)
    # x_t^T tiles, swizzled for contiguous [128, 256] per-step DMA:
    # xt[t, p, k*128+b] = x[b, t, k*128+p]
    xt = np.transpose(x, (1, 2, 0))  # [T, IN, B]
    xt = np.ascontiguousarray(
        xt.reshape(T, 2, 128, B).transpose(0, 2, 1, 3).reshape(T, 128, 2 * B)
    ).astype(BF16)

    # FC weights: out = [out_r | out_i] = h1c @ Wfc_big.T + [fcbr | fcbi]
    fcWr = np.asarray(inputs["fcWr"], np.float32)
    fcWi = np.asarray(inputs["fcWi"], np.float32)
    Wfc_big = np.block([[fcWr, -fcWi], [fcWi, fcWr]])  # [64, 2048] natural cols
    Wfc_perm = Wfc_big[:, H_PERM]  # cols follow the h layout
    # rhs sbuf layout: wfc[p, kt*64+o] = Wfc_perm[o, kt*128+p]
    wfc = np.ascontiguousarray(
        Wfc_perm.T.reshape(NHT, 128, OUT).transpose(1, 0, 2).reshape(128, NHT * OUT)
    ).astype(BF16)
    fcb = np.concatenate([inputs["fcbr"], inputs["fcbi"]]).astype(np.float32)
    fcbias = np.ascontiguousarray(np.broadcast_to(fcb, (128, OUT))).astype(np.float32)

    in_maps = []
    for s in range(NCORES):
        w0zr, w0h = _gate_weights(
            np.asarray(inputs["W0r"], np.float32),
            np.asarray(inputs["W0i"], np.float32),
            COL_PERM0,
            s,
        )
        w1zr, w1h = _gate_weights(
            np.asarray(inputs["W1r"], np.float32),
            np.asarray(inputs["W1i"], np.float32),
            COL_PERM1,
            s,
        )
        bias = np.concatenate(
            [
                _gate_biases(inputs["b0r"], inputs["b0i"], s),
                _gate_biases(inputs["b1r"], inputs["b1i"], s),
            ],
            axis=1,
        ).astype(np.float32)  # [128, 12]
        in_maps.append(
            dict(
                xt=xt,
                w0zr=_sbuf_layout(w0zr, 512).astype(BF16),
                w0h=_sbuf_layout(w0h, 256).astype(BF16),
                w1zr=_sbuf_layout(w1zr, 512).astype(BF16),
                w1h=_sbuf_layout(w1h, 256).astype(BF16),
                wfc=wfc,
                bias=np.ascontiguousarray(bias),
                fcbias=fcbias,
            )
        )
    return in_maps


# ---------------------------------------------------------------------------
# numpy emulation of the sharded algorithm (for host-side validation)
# ---------------------------------------------------------------------------


def numpy_sharded_reference(inputs, t_steps=T, dtype=np.float32):
    """Emulates exactly what the bass kernel computes (without bf16 rounding
    unless dtype=BF16-ish castings added). Used by dev tests only."""
    in_maps = host_prep(inputs)
    x = np.asarray(inputs["x"], np.float32)

    def sig(v):
        return 1.0 / (1.0 + np.exp(-v))

    def unsb(w, ncols):  # invert _sbuf_layout
        nk = w.shape[1] // ncols
        return w.reshape(128, nk, ncols).transpose(1, 0, 2).reshape(nk * 128, ncols)

    h0 = np.zeros((H, B), np.float32)  # layout rows
    h1 = np.zeros((H, B), np.float32)
    for t in range(t_steps):
        xt = x[:, t, :].T  # [256, B] = [xr, xi] rows
        c0 = np.concatenate([xt, h0], axis=0)  # [K0, B]
        # per-core gate compute
        z0 = np.zeros((H, B), np.float32)
        r0 = np.zeros((H, B), np.float32)
        for s in range(NCORES):
            wzr = unsb(in_maps[s]["w0zr"].astype(np.float32), 512)
            bias = in_maps[s]["bias"]
            pre = wzr.T @ c0  # [512, B]
            z0[s * 256 : s * 256 + 128] = sig(pre[0:128] + bias[:, 0:1])
            z0[s * 256 + 128 : s * 256 + 256] = sig(pre[128:256] + bias[:, 1:2])
            r0[s * 256 : s * 256 + 128] = sig(pre[256:384] + bias[:, 2:3])
            r0[s * 256 + 128 : s * 256 + 256] = sig(pre[384:512] + bias[:, 3:4])
        tmp0 = r0 * h0
        c0h = np.concatenate([xt, tmp0], axis=0)
        hh0 = np.zeros((H, B), np.float32)
        for s in range(NCORES):
            wh = unsb(in_maps[s]["w0h"].astype(np.float32), 256)
            bias = in_maps[s]["bias"]
            pre = wh.T @ c0h
            hh0[s * 256 : s * 256 + 128] = np.tanh(pre[0:128] + bias[:, 4:5])
            hh0[s * 256 + 128 : s * 256 + 256] = np.tanh(pre[128:256] + bias[:, 5:6])
        h0 = (1.0 - z0) * h0 + z0 * hh0
        # cell 1
        c1 = np.concatenate([h0, h1], axis=0)
        z1 = np.zeros((H, B), np.float32)
        r1 = np.zeros((H, B), np.float32)
        for s in range(NCORES):
            wzr = unsb(in_maps[s]["w1zr"].astype(np.float32), 512)
            bias = in_maps[s]["bias"]
            pre = wzr.T @ c1
            z1[s * 256 : s * 256 + 128] = sig(pre[0:128] + bias[:, 6:7])
            z1[s * 256 + 128 : s * 256 + 256] = sig(pre[128:256] + bias[:, 7:8])
            r1[s * 256 : s * 256 + 128] = sig(pre[256:384] + bias[:, 8:9])
            r1[s * 256 + 128 : s * 256 + 256] = sig(pre[384:512] + bias[:, 9:10])
        tmp1 = r1 * h1
        c1h = np.concatenate([h0, tmp1], axis=0)
        hh1 = np.zeros((H, B), np.float32)
        for s in range(NCORES):
            wh = unsb(in_maps[s]["w1h"].astype(np.float32), 256)
            bias = in_maps[s]["bias"]
            pre = wh.T @ c1h
            hh1[s * 256 : s * 256 + 128] = np.tanh(pre[0:128] + bias[:, 10:11])
            hh1[s * 256 + 128 : s * 256 + 256] = np.tanh(pre[128:256] + bias[:, 11:12])
        h1 = (1.0 - z1) * h1 + z1 * hh1
    # FC
    wfc = in_maps[0]["wfc"].astype(np.float32)
    Wfc_perm = (
        wfc.reshape(128, NHT, OUT).transpose(1, 0, 2).reshape(H, OUT).T
    )  # [64, H layout]
    out = (Wfc_perm @ h1).T + in_maps[0]["fcbias"][0]
    return out


# ---------------------------------------------------------------------------
# bass kernel
# ---------------------------------------------------------------------------


def build_kernel(t_steps=T, no_collectives=False):
    import concourse.bacc as bacc
    import concourse.mybir as mybir
    import concourse.tile as tile

    fp32 = mybir.dt.float32
    bf16 = mybir.dt.bfloat16
    AF = mybir.ActivationFunctionType

    nc = bacc.Bacc(
        "TRN2", target_bir_lowering=False, debug=False, num_devices=NCORES
    )

    d_xt = nc.dram_tensor("xt", [T, 128, 2 * B], bf16, kind="ExternalInput")
    d_w0zr = nc.dram_tensor("w0zr", [128, NK0 * 512], bf16, kind="ExternalInput")
    d_w0h = nc.dram_tensor("w0h", [128, NK0 * 256], bf16, kind="ExternalInput")
    d_w1zr = nc.dram_tensor("w1zr", [128, NK1 * 512], bf16, kind="ExternalInput")
    d_w1h = nc.dram_tensor("w1h", [128, NK1 * 256], bf16, kind="ExternalInput")
    d_wfc = nc.dram_tensor("wfc", [128, NHT * OUT], bf16, kind="ExternalInput")
    d_bias = nc.dram_tensor("bias", [128, 12], fp32, kind="ExternalInput")
    d_fcbias = nc.dram_tensor("fcbias", [128, OUT], fp32, kind="ExternalInput")
    d_out = nc.dram_tensor("out", [B, OUT], fp32, kind="ExternalOutput")

    RG = [list(range(NCORES))]

    with tile.TileContext(nc) as tc:
        with (
            tc.tile_pool(name="wpool", bufs=1) as wpool,
            tc.tile_pool(name="state", bufs=1) as spool,
            tc.tile_pool(name="work", bufs=2) as work,
            tc.tile_pool(name="xpool", bufs=3) as xpool,
            tc.tile_pool(name="pzr", bufs=2, space="PSUM") as pzr_pool,
            tc.tile_pool(name="ph", bufs=1, space="PSUM") as ph_pool,
            tc.tile_pool(name="dram", bufs=2, space="DRAM") as dram,
        ):
            # --- persistent weights -> SBUF ---
            w0zr = wpool.tile([128, NK0 * 512], bf16)
            w0h = wpool.tile([128, NK0 * 256], bf16)
            w1zr = wpool.tile([128, NK1 * 512], bf16)
            w1h = wpool.tile([128, NK1 * 256], bf16)
            wfc = wpool.tile([128, NHT * OUT], bf16)
            bias = wpool.tile([128, 12], fp32)
            fcbias = wpool.tile([128, OUT], fp32)
            for dst, src in [
                (w0zr, d_w0zr),
                (w0h, d_w0h),
                (w1zr, d_w1zr),
                (w1h, d_w1h),
                (wfc, d_wfc),
                (bias, d_bias),
                (fcbias, d_fcbias),
            ]:
                nc.sync.dma_start(dst[:], src[:])

            def mm_group(psum, wsb, njt, wstride, rhs_entries):
                """psum [128, njt*128]; wsb stationary; rhs_entries =
                [(k_global, rhs_ap)]; contraction accumulated per j tile."""
                last = len(rhs_entries) - 1
                for j in range(njt):
                    for idx, (kg, rap) in enumerate(rhs_entries):
                        nc.tensor.matmul(
                            psum[:, j * 128 : (j + 1) * 128],
                            wsb[:, kg * wstride + j * 128 : kg * wstride + (j + 1) * 128],
                            rap,
                            start=(idx == 0),
                            stop=(idx == last),
                        )

            def hf_entries(hf, base):
                return [(base + i, hf[:, i * 128 : (i + 1) * 128]) for i in range(NHT)]

            # state buffers (python-managed ping-pong)
            h0f = [None, None]  # full h0 (bf16, [128, H])
            h1f = [None, None]
            h0loc = [None, None]  # own fp32 shard [128, 256]
            h1loc = [None, None]

            def new_state_tiles(i):
                h0f[i] = spool.tile([128, H], bf16, name=f"h0f{i}")
                h1f[i] = spool.tile([128, H], bf16, name=f"h1f{i}")
                h0loc[i] = spool.tile([128, 256], fp32, name=f"h0loc{i}")
                h1loc[i] = spool.tile([128, 256], fp32, name=f"h1loc{i}")

            new_state_tiles(0)
            new_state_tiles(1)

            def emit_gates_zr(psum, z_t, r_t, bcol):
                # z fp32 (cols 0:256 of psum), r bf16 (cols 256:512)
                nc.scalar.activation(
                    z_t[:, 0:128], psum[:, 0:128], AF.Sigmoid,
                    bias=bias[:, bcol : bcol + 1],
                )
                nc.scalar.activation(
                    z_t[:, 128:256], psum[:, 128:256], AF.Sigmoid,
                    bias=bias[:, bcol + 1 : bcol + 2],
                )
                nc.scalar.activation(
                    r_t[:, 0:128], psum[:, 256:384], AF.Sigmoid,
                    bias=bias[:, bcol + 2 : bcol + 3],
                )
                nc.scalar.activation(
                    r_t[:, 128:256], psum[:, 384:512], AF.Sigmoid,
                    bias=bias[:, bcol + 3 : bcol + 4],
                )

            def emit_tanh(psum, hh_t, bcol):
                nc.scalar.activation(
                    hh_t[:, 0:128], psum[:, 0:128], AF.Tanh,
                    bias=bias[:, bcol : bcol + 1],
                )
                nc.scalar.activation(
                    hh_t[:, 128:256], psum[:, 128:256], AF.Tanh,
                    bias=bias[:, bcol + 1 : bcol + 2],
                )

            def ag_shard(shard_bf, tag):
                """DMA shard [128, 256] -> DRAM [256, 128], AllGather, return
                the gathered DRAM tile [H, 128]."""
                agin = dram.tile([256, B], bf16, name=f"agin_{tag}")
                agout = dram.tile(
                    [H, B], bf16, name=f"agout_{tag}",
                    addr_space="Local" if no_collectives else "Shared",
                )
                nc.gpsimd.dma_start(
                    agin[:].rearrange("(a p) b -> p a b", p=128),
                    shard_bf[:].rearrange("p (a b) -> p a b", b=B),
                )
                if no_collectives:
                    # timing-diagnostic variant: same data volume, no comm
                    for s in range(NCORES):
                        nc.sync.dma_start(
                            agout[s * 256 : (s + 1) * 256, :], agin[:]
                        )
                else:
                    nc.gpsimd.collective_compute(
                        "AllGather",
                        mybir.AluOpType.bypass,
                        replica_groups=RG,
                        ins=[agin[:]],
                        outs=[agout[:]],
                    )
                return agout

            def dma_gather_in(hf, agout):
                # 2 strided DMAs (8 h-tiles each) to balance dispatch cost
                # on the Sync queue vs per-engine transfer parallelism
                half = NHT // 2
                for i in range(2):
                    nc.sync.dma_start(
                        hf[:, i * half * 128 : (i + 1) * half * 128].rearrange(
                            "p (n b) -> p n b", b=B
                        ),
                        agout[i * half * 128 : (i + 1) * half * 128, :].rearrange(
                            "(n p) b -> p n b", p=128
                        ),
                    )

            def state_update(loc_prev, loc_new, z_t, hh_t, first):
                if first:
                    nc.vector.tensor_mul(loc_new[:], z_t[:], hh_t[:])
                else:
                    d = work.tile([128, 256], fp32, name="upd_d")
                    e = work.tile([128, 256], fp32, name="upd_e")
                    nc.vector.tensor_sub(d[:], hh_t[:], loc_prev[:])
                    nc.vector.tensor_mul(e[:], z_t[:], d[:])
                    nc.vector.tensor_add(loc_new[:], loc_prev[:], e[:])

            # ---------------- main pipelined loop ----------------
            # slot t: cell0(t) interleaved with cell1(t-1)
            pend_ag_h0 = None  # AG of h0'(t-1), to land in h0f[cur]
            pend_ag_h1 = None  # AG of h1'(t-2), to land in h1f[cur]

            for t in range(t_steps + 1):
                cur = t % 2
                prv = 1 - cur

                # land pending gathers for this slot
                if pend_ag_h0 is not None:
                    dma_gather_in(h0f[cur], pend_ag_h0)
                    pend_ag_h0 = None
                if pend_ag_h1 is not None:
                    dma_gather_in(h1f[cur], pend_ag_h1)
                    pend_ag_h1 = None

                ag_r0 = None
                z0_t = hh0_t = None
                if t < t_steps:
                    # ---- Phase A: cell0(t) z/r ----
                    xt_t = xpool.tile([128, 2 * B], bf16, name="xt_t")
                    nc.sync.dma_start(xt_t[:], d_xt[t])
                    x_entries = [(0, xt_t[:, 0:128]), (1, xt_t[:, 128:256])]
                    p0zr = pzr_pool.tile([128, 512], fp32, name="p0zr")
                    ents = list(x_entries)
                    if t > 0:
                        ents += hf_entries(h0f[cur], 2)
                    mm_group(p0zr, w0zr, 4, 512, ents)
                    z0_t = work.tile([128, 256], fp32, name="z0_t")
                    r0_sh = work.tile([128, 256], bf16, name="r0_sh")
                    emit_gates_zr(p0zr, z0_t, r0_sh, 0)
                    ag_r0 = ag_shard(r0_sh, "r0")

                ag_r1 = None
                z1_t = None
                if 1 <= t:
                    # ---- Phase B: cell1(t-1) z/r ----
                    p1zr = pzr_pool.tile([128, 512], fp32, name="p1zr")
                    ents = hf_entries(h0f[cur], 0)
                    if t > 1:
                        ents += hf_entries(h1f[cur], NHT)
                    mm_group(p1zr, w1zr, 4, 512, ents)
                    z1_t = work.tile([128, 256], fp32, name="z1_t")
                    r1_sh = work.tile([128, 256], bf16, name="r1_sh")
                    emit_gates_zr(p1zr, z1_t, r1_sh, 6)
                    ag_r1 = ag_shard(r1_sh, "r1")

                if t < t_steps:
                    # ---- Phase C: cell0(t) candidate + update ----
                    ents = [(0, xt_t[:, 0:128]), (1, xt_t[:, 128:256])]
                    if t > 0:
                        r0f = work.tile([128, H], bf16, name="r0f")
                        dma_gather_in(r0f, ag_r0)
                        tmp0 = work.tile([128, H], bf16, name="tmp0")
                        nc.vector.tensor_mul(tmp0[:], r0f[:], h0f[cur][:])
                        ents += hf_entries(tmp0, 2)
                    p0h = ph_pool.tile([128, 256], fp32, name="p0h")
                    mm_group(p0h, w0h, 2, 256, ents)
                    hh0_t = work.tile([128, 256], fp32, name="hh0_t")
                    emit_tanh(p0h, hh0_t, 4)
                    state_update(h0loc[prv], h0loc[cur], z0_t, hh0_t, first=(t == 0))
                    h0_sh = work.tile([128, 256], bf16, name="h0_sh")
                    nc.vector.tensor_copy(h0_sh[:], h0loc[cur][:])
                    pend_ag_h0 = ag_shard(h0_sh, "h0")

                if 1 <= t:
                    # ---- Phase D: cell1(t-1) candidate + update ----
                    ents = hf_entries(h0f[cur], 0)
                    if t > 1:
                        r1f = work.tile([128, H], bf16, name="r1f")
                        dma_gather_in(r1f, ag_r1)
                        tmp1 = work.tile([128, H], bf16, name="tmp1")
                        nc.vector.tensor_mul(tmp1[:], r1f[:], h1f[cur][:])
                        ents += hf_entries(tmp1, NHT)
                    p1h = ph_pool.tile([128, 256], fp32, name="p1h")
                    mm_group(p1h, w1h, 2, 256, ents)
                    hh1_t = work.tile([128, 256], fp32, name="hh1_t")
                    emit_tanh(p1h, hh1_t, 10)
                    state_update(h1loc[prv], h1loc[cur], z1_t, hh1_t, first=(t == 1))
                    h1_sh = work.tile([128, 256], bf16, name="h1_sh")
                    nc.vector.tensor_copy(h1_sh[:], h1loc[cur][:])
                    pend_ag_h1 = ag_shard(h1_sh, "h1")

                if t < t_steps:
                    new_state_tiles(prv)  # rotate buffers for next slot

            # ---------------- final FC ----------------
            # last pending AG: h1'(T-1) full
            h1_final = spool.tile([128, H], bf16, name="h1_final")
            dma_gather_in(h1_final, pend_ag_h1)
            pfc = ph_pool.tile([128, OUT], fp32, name="pfc")
            for kt in range(NHT):
                nc.tensor.matmul(
                    pfc[:],
                    h1_final[:, kt * 128 : (kt + 1) * 128],
                    wfc[:, kt * OUT : (kt + 1) * OUT],
                    start=(kt == 0),
                    stop=(kt == NHT - 1),
                )
            out_sb = work.tile([128, OUT], fp32, name="out_sb")
            nc.vector.tensor_add(out_sb[:], pfc[:], fcbias[:])
            nc.sync.dma_start(d_out[:], out_sb[:])

    nc.compile()
    return nc


_CACHE = {}


def _get_nc(t_steps=T, no_collectives=False):
    key = (t_steps, no_collectives)
    if key not in _CACHE:
        _CACHE[key] = build_kernel(t_steps, no_collectives=no_collectives)
    return _CACHE[key]


def run(inputs, t_steps=T, trace=False):
    from concourse import bass_utils

    nc = _get_nc(t_steps)
    in_maps = host_prep(inputs)
    res = bass_utils.run_bass_kernel_spmd(
        nc, in_maps, core_ids=list(range(NCORES)), trace=trace
    )
    out = np.asarray(res.results[0]["out"], np.float32)
    return out, res


def timed_run(inputs, t_steps=T, iters=4, no_collectives=False, measure_reps=False):
    """Execute via PJRT with a persistent jitted executable; time each call."""
    import time

    import jax
    from jax.sharding import Mesh, PartitionSpec
    from jax.experimental.shard_map import shard_map

    import concourse.mybir as mybir
    from concourse import bass2jax

    nc = _get_nc(t_steps, no_collectives=no_collectives)
    in_maps = host_prep(inputs)
    bass2jax.install_neuronx_cc_hook()

    partition_name = nc.partition_id_tensor.name if nc.partition_id_tensor else None
    in_names, out_names, out_avals, zero_outs = [], [], [], []
    for alloc in nc.m.functions[0].allocations:
        if not isinstance(alloc, mybir.MemoryLocationSet):
            continue
        name = alloc.memorylocations[0].name
        if alloc.kind == "ExternalInput":
            if name != partition_name:
                in_names.append(name)
        elif alloc.kind == "ExternalOutput":
            out_names.append(name)
            shape = tuple(alloc.tensor_shape)
            dtype = mybir.dt.np(alloc.dtype)
            out_avals.append(jax.core.ShapedArray(shape, dtype))
            zero_outs.append(np.zeros(shape, dtype))
    n_params = len(in_names)
    n_outs = len(out_avals)
    all_in_names = list(in_names) + list(out_names)
    if partition_name is not None:
        all_in_names = all_in_names + [partition_name]

    def _body(*args):
        operands = list(args)
        if partition_name is not None:
            operands.append(bass2jax.partition_id_tensor())
        outs = bass2jax._bass_exec_p.bind(
            *operands,
            out_avals=tuple(out_avals),
            in_names=tuple(all_in_names),
            out_names=tuple(out_names),
            lowering_input_output_aliases=(),
            sim_require_finite=True,
            sim_require_nnan=True,
            nc=nc,
        )
        return tuple(outs)

    devices = jax.devices()[:NCORES]
    mesh = Mesh(np.asarray(devices), ("core",))
    in_specs = (PartitionSpec("core"),) * (n_params + n_outs)
    out_specs = (PartitionSpec("core"),) * n_outs
    donate = tuple(range(n_params, n_params + n_outs))
    sharded = jax.jit(
        shard_map(
            _body, mesh=mesh, in_specs=in_specs, out_specs=out_specs, check_rep=False
        ),
        donate_argnums=donate,
        keep_unused=True,
    )
    per_core = [[np.asarray(m[name]) for name in in_names] for m in in_maps]
    concat_in = [
        np.concatenate([per_core[c][i] for c in range(NCORES)], axis=0)
        for i in range(n_params)
    ]
    sharding = jax.sharding.NamedSharding(mesh, PartitionSpec("core"))
    dev_in = [jax.device_put(a, sharding) for a in concat_in]

    def one_call():
        zeros = [
            jax.device_put(
                np.zeros((NCORES * z.shape[0], *z.shape[1:]), z.dtype), sharding
            )
            for z in zero_outs
        ]
        for z in zeros:
            z.block_until_ready()
        t0 = time.perf_counter()
        outs = sharded(*dev_in, *zeros)
        for o in outs:
            o.block_until_ready()
        return time.perf_counter() - t0, outs

    times = []
    outs = None
    for _ in range(iters):
        dt, outs = one_call()
        times.append(dt)
    out0 = np.asarray(outs[0]).reshape(NCORES, *out_avals[0].shape)[0]

    return dict(times=times, best=min(times), out=np.asarray(out0, np.float32))


def kernel(**inputs):
    out, _ = run(inputs)
    return out



# revision 4
# speedup vs baseline: 1.2317x; 1.2317x over previous
"""Trainium2 Bass kernel for a 2-layer complex-gated GRU.

Problem (hardcoded):
  B=128, T=128, IN=256, H=2048, OUT=64, fp32.
  reference: 2 stacked complex GRU cells scanned over T, then a complex FC.

Strategy: 8-way tensor-parallel over the gate output dim (each core owns a
128-wide slice of the 1024 complex gate outputs => 256 rows of the 2048-row
real-valued hidden state).

v2: activation-stationary matmuls. The stationary (LDWEIGHTS) operand is an
activation tile [128 contraction-features, 128 batch]; the moving operand is
the weight block [128 features, N out-cols] with N=512 (z/r gates) or N=256
(candidate), so LDWEIGHTS overhead hides under long moving streams (100
matmuls/step vs 300 in the weight-stationary form). Matmul outputs land
batch-major [B, out]; gates and the state update run batch-major on the
core-local fp32 shard. Only the 256-wide shard to be all-gathered (tmp=r*h,
and h') is transposed back to feature-major (2 PE transposes) before the AG.
Per-step collectives: 4 all-gathers (tmp0, tmp1, h0', h1'), software-pipelined
across the two cells (cell0 at t runs with cell1 at t-1). Matmul operands are
bf16, accumulation fp32 in PSUM, state update in fp32.
"""

import functools
import os

import numpy as np
import ml_dtypes

B, T, IN, H, OUT = 128, 128, 256, 2048, 64
NCORES = 8
M = H // 2  # 1024 complex gate outputs per gate
SH = M // NCORES  # 128 complex outputs per core per gate
K0 = IN + H  # 2304 = layer-0 contraction length (real-valued)
K1 = H + H  # 4096 = layer-1 contraction length
NK0 = K0 // 128  # 18
NK1 = K1 // 128  # 32
NHT = H // 128  # 16 tiles of the hidden state

BF16 = ml_dtypes.bfloat16


def _h_perm():
    """h storage layout: row p holds, for shard s=p//256, q=p%256:
    q<128 -> real component s*128+q ; q>=128 -> imag component s*128+(q-128).
    Returns perm such that h_layout[p] = h_natural[perm[p]] with
    h_natural = [hr (1024), hi (1024)]."""
    p = np.arange(H)
    s = p // 256
    q = p % 256
    comp = s * 128 + (q % 128)
    return np.where(q < 128, comp, M + comp)


H_PERM = _h_perm()


def _col_perm_layer0():
    """Contraction row t -> column of A=[cr-block | ci-block] (width 2*n0).
    Row layout: [xr (128), xi (128), h in H_PERM layout (2048)].
    cr = [xr(128), hr(1024)], ci = [xi(128), hi(1024)]."""
    n0 = K0 // 2  # 1152
    ih = IN // 2  # 128
    cols = np.empty(K0, np.int64)
    t = np.arange(K0)
    cols[:ih] = t[:ih]
    cols[ih : 2 * ih] = n0 + (t[ih : 2 * ih] - ih)
    p = t[2 * ih :] - 2 * ih
    comp = H_PERM[p]
    is_real = comp < M
    c = np.where(is_real, comp, comp - M)
    cols[2 * ih :] = np.where(is_real, ih + c, n0 + ih + c)
    return cols


def _col_perm_layer1():
    """Row layout: [h0 in H_PERM layout (2048), h1 in H_PERM layout (2048)].
    'x' of cell1 is h0. cr = [h0r(1024), h1r(1024)], ci = [h0i, h1i]."""
    n1 = K1 // 2  # 2048
    ih = H // 2  # 1024
    cols = np.empty(K1, np.int64)
    for blk in range(2):  # 0: h0 (x-part), 1: h1 (h-part)
        p = np.arange(H)
        comp = H_PERM[p]
        is_real = comp < M
        c = np.where(is_real, comp, comp - M)
        base_r = blk * ih
        base_i = n1 + blk * ih
        cols[blk * H : (blk + 1) * H] = np.where(is_real, base_r + c, base_i + c)
    return cols


COL_PERM0 = _col_perm_layer0()
COL_PERM1 = _col_perm_layer1()


def _sbuf_layout(w, ncols):
    """[K, ncols] -> [128, (K//128)*ncols] with block k at cols [k*ncols:(k+1)*ncols]."""
    K = w.shape[0]
    nk = K // 128
    return np.ascontiguousarray(
        w.reshape(nk, 128, ncols).transpose(1, 0, 2).reshape(128, nk * ncols)
    )


def _gate_weights(Wr, Wi, col_perm, s):
    """Per-core weights for one layer.

    Wr/Wi: [3, M, n] (gates z, r, h~). Returns (wzr [K,512], wh [K,256]) where
    output col blocks are [z_r, z_i, r_r, r_i] and [h_r, h_i], each 128 wide
    (core s's slice), and rows follow col_perm's contraction layout."""
    sl = slice(s * SH, (s + 1) * SH)
    blocks_zr = []
    for g in range(2):
        A_r = np.hstack([Wr[g], -Wi[g]])  # real output rows
        A_i = np.hstack([Wi[g], Wr[g]])  # imag output rows
        blocks_zr.append(A_r[sl][:, col_perm].T)
        blocks_zr.append(A_i[sl][:, col_perm].T)
    wzr = np.concatenate(blocks_zr, axis=1)
    A_r = np.hstack([Wr[2], -Wi[2]])
    A_i = np.hstack([Wi[2], Wr[2]])
    wh = np.concatenate([A_r[sl][:, col_perm].T, A_i[sl][:, col_perm].T], axis=1)
    return wzr, wh


def host_prep(inputs):
    """Build per-core in_maps (numpy) from the full problem inputs."""
    x = np.asarray(inputs["x"], np.float32)
    # x_t^T tiles, swizzled for contiguous [128, 256] per-step DMA:
    # xt[t, p, k*128+b] = x[b, t, k*128+p]
    xt = np.transpose(x, (1, 2, 0))  # [T, IN, B]
    xt = np.ascontiguousarray(
        xt.reshape(T, 2, 128, B).transpose(0, 2, 1, 3).reshape(T, 128, 2 * B)
    ).astype(BF16)

    # FC weights: out = [out_r | out_i] = h1c @ Wfc_big.T + [fcbr | fcbi]
    fcWr = np.asarray(inputs["fcWr"], np.float32)
    fcWi = np.asarray(inputs["fcWi"], np.float32)
    Wfc_big = np.block([[fcWr, -fcWi], [fcWi, fcWr]])  # [64, 2048] natural cols
    Wfc_perm = Wfc_big[:, H_PERM]  # cols follow the h layout
    wfc = np.ascontiguousarray(
        Wfc_perm.T.reshape(NHT, 128, OUT).transpose(1, 0, 2).reshape(128, NHT * OUT)
    ).astype(BF16)
    fcb = np.concatenate([inputs["fcbr"], inputs["fcbi"]]).astype(np.float32)
    fcbias = np.ascontiguousarray(np.broadcast_to(fcb, (128, OUT))).astype(np.float32)

    b0r = np.asarray(inputs["b0r"], np.float32)
    b0i = np.asarray(inputs["b0i"], np.float32)
    b1r = np.asarray(inputs["b1r"], np.float32)
    b1i = np.asarray(inputs["b1i"], np.float32)

    in_maps = []
    for s in range(NCORES):
        sl = slice(s * SH, (s + 1) * SH)
        w0zr, w0h = _gate_weights(
            np.asarray(inputs["W0r"], np.float32),
            np.asarray(inputs["W0i"], np.float32),
            COL_PERM0,
            s,
        )
        w1zr, w1h = _gate_weights(
            np.asarray(inputs["W1r"], np.float32),
            np.asarray(inputs["W1i"], np.float32),
            COL_PERM1,
            s,
        )
        # bias as a row vector, replicated across partitions (outputs are
        # batch-major): [c0 z_r|z_i|r_r|r_i (512), c1 same (512),
        #                c0 h_r|h_i (256), c1 h_r|h_i (256)]
        bias_vec = np.concatenate(
            [
                b0r[0][sl], b0i[0][sl], b0r[1][sl], b0i[1][sl],
                b1r[0][sl], b1i[0][sl], b1r[1][sl], b1i[1][sl],
                b0r[2][sl], b0i[2][sl],
                b1r[2][sl], b1i[2][sl],
            ]
        ).astype(np.float32)  # [1536]
        biasf = np.ascontiguousarray(np.broadcast_to(bias_vec, (128, 1536)))
        in_maps.append(
            dict(
                xt=xt,
                w0zr=_sbuf_layout(w0zr, 512).astype(BF16),
                w0h=_sbuf_layout(w0h, 256).astype(BF16),
                w1zr=_sbuf_layout(w1zr, 512).astype(BF16),
                w1h=_sbuf_layout(w1h, 256).astype(BF16),
                wfc=wfc,
                biasf=biasf,
                fcbias=fcbias,
            )
        )
    return in_maps


# ---------------------------------------------------------------------------
# numpy emulation of the sharded algorithm (for host-side validation)
# ---------------------------------------------------------------------------


def numpy_sharded_reference(inputs, t_steps=T):
    """Emulates what the bass kernel computes (without bf16 rounding)."""
    in_maps = host_prep(inputs)
    x = np.asarray(inputs["x"], np.float32)

    def sig(v):
        return 1.0 / (1.0 + np.exp(-v))

    def unsb(w, ncols):  # invert _sbuf_layout
        nk = w.shape[1] // ncols
        return w.reshape(128, nk, ncols).transpose(1, 0, 2).reshape(nk * 128, ncols)

    h0 = np.zeros((H, B), np.float32)  # layout rows (feature-major)
    h1 = np.zeros((H, B), np.float32)
    for t in range(t_steps):
        xt = x[:, t, :].T  # [256, B] = [xr, xi] rows
        c0 = np.concatenate([xt, h0], axis=0)  # [K0, B]
        z0 = np.zeros((H, B), np.float32)
        r0 = np.zeros((H, B), np.float32)
        for s in range(NCORES):
            wzr = unsb(in_maps[s]["w0zr"].astype(np.float32), 512)
            bias = in_maps[s]["biasf"][0]
            pre = wzr.T @ c0  # [512, B]
            z0[s * 256 : s * 256 + 256] = sig(pre[0:256] + bias[0:256, None])
            r0[s * 256 : s * 256 + 256] = sig(pre[256:512] + bias[256:512, None])
        tmp0 = r0 * h0
        c0h = np.concatenate([xt, tmp0], axis=0)
        hh0 = np.zeros((H, B), np.float32)
        for s in range(NCORES):
            wh = unsb(in_maps[s]["w0h"].astype(np.float32), 256)
            bias = in_maps[s]["biasf"][0]
            pre = wh.T @ c0h
            hh0[s * 256 : s * 256 + 256] = np.tanh(pre + bias[1024:1280, None])
        h0 = (1.0 - z0) * h0 + z0 * hh0
        # cell 1
        c1 = np.concatenate([h0, h1], axis=0)
        z1 = np.zeros((H, B), np.float32)
        r1 = np.zeros((H, B), np.float32)
        for s in range(NCORES):
            wzr = unsb(in_maps[s]["w1zr"].astype(np.float32), 512)
            bias = in_maps[s]["biasf"][0]
            pre = wzr.T @ c1
            z1[s * 256 : s * 256 + 256] = sig(pre[0:256] + bias[512:768, None])
            r1[s * 256 : s * 256 + 256] = sig(pre[256:512] + bias[768:1024, None])
        tmp1 = r1 * h1
        c1h = np.concatenate([h0, tmp1], axis=0)
        hh1 = np.zeros((H, B), np.float32)
        for s in range(NCORES):
            wh = unsb(in_maps[s]["w1h"].astype(np.float32), 256)
            bias = in_maps[s]["biasf"][0]
            pre = wh.T @ c1h
            hh1[s * 256 : s * 256 + 256] = np.tanh(pre + bias[1280:1536, None])
        h1 = (1.0 - z1) * h1 + z1 * hh1
    wfc = in_maps[0]["wfc"].astype(np.float32)
    Wfc_perm = (
        wfc.reshape(128, NHT, OUT).transpose(1, 0, 2).reshape(H, OUT).T
    )
    out = (Wfc_perm @ h1).T + in_maps[0]["fcbias"][0]
    return out


# ---------------------------------------------------------------------------
# bass kernel
# ---------------------------------------------------------------------------


def build_kernel(t_steps=T, no_collectives=False):
    import concourse.bacc as bacc
    import concourse.mybir as mybir
    import concourse.tile as tile
    from concourse import masks

    fp32 = mybir.dt.float32
    bf16 = mybir.dt.bfloat16
    AF = mybir.ActivationFunctionType

    nc = bacc.Bacc(
        "TRN2", target_bir_lowering=False, debug=False, num_devices=NCORES
    )

    d_xt = nc.dram_tensor("xt", [T, 128, 2 * B], bf16, kind="ExternalInput")
    d_w0zr = nc.dram_tensor("w0zr", [128, NK0 * 512], bf16, kind="ExternalInput")
    d_w0h = nc.dram_tensor("w0h", [128, NK0 * 256], bf16, kind="ExternalInput")
    d_w1zr = nc.dram_tensor("w1zr", [128, NK1 * 512], bf16, kind="ExternalInput")
    d_w1h = nc.dram_tensor("w1h", [128, NK1 * 256], bf16, kind="ExternalInput")
    d_wfc = nc.dram_tensor("wfc", [128, NHT * OUT], bf16, kind="ExternalInput")
    d_biasf = nc.dram_tensor("biasf", [128, 1536], fp32, kind="ExternalInput")
    d_fcbias = nc.dram_tensor("fcbias", [128, OUT], fp32, kind="ExternalInput")
    d_out = nc.dram_tensor("out", [B, OUT], fp32, kind="ExternalOutput")

    RG = [list(range(NCORES))]

    with tile.TileContext(nc) as tc:
        with (
            tc.tile_pool(name="wpool", bufs=1) as wpool,
            tc.tile_pool(name="state", bufs=1) as spool,
            tc.tile_pool(name="work", bufs=2) as work,
            tc.tile_pool(name="xpool", bufs=3) as xpool,
            tc.tile_pool(name="pzr", bufs=1, space="PSUM") as pzr_pool,
            tc.tile_pool(name="ph", bufs=1, space="PSUM") as ph_pool,
            tc.tile_pool(name="ptp", bufs=1, space="PSUM") as ptp_pool,
            tc.tile_pool(name="dram", bufs=2, space="DRAM") as dram,
        ):
            # --- persistent weights -> SBUF ---
            w0zr = wpool.tile([128, NK0 * 512], bf16)
            w0h = wpool.tile([128, NK0 * 256], bf16)
            w1zr = wpool.tile([128, NK1 * 512], bf16)
            w1h = wpool.tile([128, NK1 * 256], bf16)
            wfc = wpool.tile([128, NHT * OUT], bf16)
            biasf = wpool.tile([128, 1536], fp32)
            fcbias = wpool.tile([128, OUT], fp32)
            for dst, src in [
                (w0zr, d_w0zr),
                (w0h, d_w0h),
                (w1zr, d_w1zr),
                (w1h, d_w1h),
                (wfc, d_wfc),
                (biasf, d_biasf),
                (fcbias, d_fcbias),
            ]:
                nc.sync.dma_start(dst[:], src[:])
            ident = wpool.tile([128, 128], bf16)
            masks.make_identity(nc, ident[:])

            def mm_group(psum, wsb, ncols, lhs_entries):
                """psum [128(B), ncols]; lhs_entries = [(k_global, act_ap)]
                stationary activations; moving weight block per k tile."""
                last = len(lhs_entries) - 1
                for idx, (kg, lap) in enumerate(lhs_entries):
                    nc.tensor.matmul(
                        psum[:, 0:ncols],
                        lap,
                        wsb[:, kg * ncols : (kg + 1) * ncols],
                        start=(idx == 0),
                        stop=(idx == last),
                    )

            def hf_entries(hf, base):
                return [(base + i, hf[:, i * 128 : (i + 1) * 128]) for i in range(NHT)]

            # state buffers (python-managed ping-pong)
            h0f = [None, None]  # full h0, feature-major (bf16, [128, H])
            h1f = [None, None]
            h0loc = [None, None]  # own fp32 shard, batch-major [128(B), 256]
            h1loc = [None, None]

            def new_state_tiles(i):
                h0f[i] = spool.tile([128, H], bf16, name=f"h0f{i}")
                h1f[i] = spool.tile([128, H], bf16, name=f"h1f{i}")
                h0loc[i] = spool.tile([128, 256], fp32, name=f"h0loc{i}")
                h1loc[i] = spool.tile([128, 256], fp32, name=f"h1loc{i}")

            new_state_tiles(0)
            new_state_tiles(1)

            def emit_zr(psum, bcol, tag):
                """psum [128,512] -> zr=sigmoid(psum+bias) fp32 [128,512].
                z = zr[:, 0:256], r = zr[:, 256:512]."""
                nc.vector.tensor_add(psum[:], psum[:], biasf[:, bcol : bcol + 512])
                zr = work.tile([128, 512], fp32, name=f"zr_{tag}")
                nc.scalar.activation(zr[:], psum[:], AF.Sigmoid)
                return zr

            def emit_h(psum, bcol, tag):
                nc.vector.tensor_add(psum[:], psum[:], biasf[:, bcol : bcol + 256])
                hh = work.tile([128, 256], fp32, name=f"hh_{tag}")
                nc.scalar.activation(hh[:], psum[:], AF.Tanh)
                return hh

            def ag_shard(shard_bf, tag):
                """shard_bf: batch-major bf16 [128(B), 256]. Transpose to
                feature-major [256, B] via PE, DMA to DRAM, AllGather.
                Returns gathered DRAM tile [H, B]."""
                pt = ptp_pool.tile(
                    [128, 256], bf16, name="pt_tmp" if "tmp" in tag else "pt_h"
                )
                nc.tensor.transpose(pt[:, 0:128], shard_bf[:, 0:128], ident[:])
                nc.tensor.transpose(pt[:, 128:256], shard_bf[:, 128:256], ident[:])
                st = work.tile([128, 256], bf16, name=f"st_{tag}")
                nc.vector.tensor_copy(st[:], pt[:])
                agin = dram.tile([256, B], bf16, name=f"agin_{tag}")
                agout = dram.tile(
                    [H, B], bf16, name=f"agout_{tag}",
                    addr_space="Local" if no_collectives else "Shared",
                )
                nc.gpsimd.dma_start(
                    agin[:].rearrange("(a p) b -> p a b", p=128),
                    st[:].rearrange("p (a b) -> p a b", b=B),
                )
                if no_collectives:
                    for s in range(NCORES):
                        nc.sync.dma_start(
                            agout[s * 256 : (s + 1) * 256, :], agin[:]
                        )
                else:
                    nc.gpsimd.collective_compute(
                        "AllGather",
                        mybir.AluOpType.bypass,
                        replica_groups=RG,
                        ins=[agin[:]],
                        outs=[agout[:]],
                    )
                return agout

            def dma_gather_in(hf, agout):
                half = NHT // 2
                for i in range(2):
                    nc.sync.dma_start(
                        hf[:, i * half * 128 : (i + 1) * half * 128].rearrange(
                            "p (n b) -> p n b", b=B
                        ),
                        agout[i * half * 128 : (i + 1) * half * 128, :].rearrange(
                            "(n p) b -> p n b", p=128
                        ),
                    )

            def state_update(loc_prev, loc_new, z_ap, hh_t, first):
                if first:
                    nc.vector.tensor_mul(loc_new[:], z_ap, hh_t[:])
                else:
                    d = work.tile([128, 256], fp32, name="upd_d")
                    e = work.tile([128, 256], fp32, name="upd_e")
                    nc.vector.tensor_sub(d[:], hh_t[:], loc_prev[:])
                    nc.vector.tensor_mul(e[:], z_ap, d[:])
                    nc.vector.tensor_add(loc_new[:], loc_prev[:], e[:])

            # ---------------- main pipelined loop ----------------
            # slot t: cell0(t) interleaved with cell1(t-1)
            pend_ag_h0 = None  # AG of h0'(t-1), to land in h0f[cur]
            pend_ag_h1 = None  # AG of h1'(t-2), to land in h1f[cur]

            for t in range(t_steps + 1):
                cur = t % 2
                prv = 1 - cur

                if pend_ag_h0 is not None:
                    dma_gather_in(h0f[cur], pend_ag_h0)
                    pend_ag_h0 = None
                if pend_ag_h1 is not None:
                    dma_gather_in(h1f[cur], pend_ag_h1)
                    pend_ag_h1 = None

                ag_tmp0 = None
                zr0 = None
                if t < t_steps:
                    # ---- Phase A: cell0(t) z/r ----
                    xt_t = xpool.tile([128, 2 * B], bf16, name="xt_t")
                    nc.sync.dma_start(xt_t[:], d_xt[t])
                    x_entries = [(0, xt_t[:, 0:128]), (1, xt_t[:, 128:256])]
                    p0zr = pzr_pool.tile([128, 512], fp32, name="p0zr")
                    ents = list(x_entries)
                    if t > 0:
                        ents += hf_entries(h0f[cur], 2)
                    mm_group(p0zr, w0zr, 512, ents)
                    zr0 = emit_zr(p0zr, 0, "zr0")
                    if t > 0:
                        tmp0 = work.tile([128, 256], bf16, name="tmp0")
                        nc.vector.tensor_mul(
                            tmp0[:], zr0[:, 256:512], h0loc[prv][:]
                        )
                        ag_tmp0 = ag_shard(tmp0, "tmp0")

                ag_tmp1 = None
                zr1 = None
                if 1 <= t:
                    # ---- Phase B: cell1(t-1) z/r ----
                    p1zr = pzr_pool.tile([128, 512], fp32, name="p1zr")
                    ents = hf_entries(h0f[cur], 0)
                    if t > 1:
                        ents += hf_entries(h1f[cur], NHT)
                    mm_group(p1zr, w1zr, 512, ents)
                    zr1 = emit_zr(p1zr, 512, "zr1")
                    if t > 1:
                        tmp1 = work.tile([128, 256], bf16, name="tmp1")
                        nc.vector.tensor_mul(
                            tmp1[:], zr1[:, 256:512], h1loc[prv][:]
                        )
                        ag_tmp1 = ag_shard(tmp1, "tmp1")

                if t < t_steps:
                    # ---- Phase C: cell0(t) candidate + update ----
                    ents = [(0, xt_t[:, 0:128]), (1, xt_t[:, 128:256])]
                    if t > 0:
                        tmp0f = work.tile([128, H], bf16, name="tmp0f")
                        dma_gather_in(tmp0f, ag_tmp0)
                        ents += hf_entries(tmp0f, 2)
                    p0h = ph_pool.tile([128, 256], fp32, name="p0h")
                    mm_group(p0h, w0h, 256, ents)
                    hh0 = emit_h(p0h, 1024, "hh0")
                    state_update(
                        h0loc[prv], h0loc[cur], zr0[:, 0:256], hh0, first=(t == 0)
                    )
                    h0_sh = work.tile([128, 256], bf16, name="h0_sh")
                    nc.vector.tensor_copy(h0_sh[:], h0loc[cur][:])
                    pend_ag_h0 = ag_shard(h0_sh, "h0")

                if 1 <= t:
                    # ---- Phase D: cell1(t-1) candidate + update ----
                    ents = hf_entries(h0f[cur], 0)
                    if t > 1:
                        tmp1f = work.tile([128, H], bf16, name="tmp1f")
                        dma_gather_in(tmp1f, ag_tmp1)
                        ents += hf_entries(tmp1f, NHT)
                    p1h = ph_pool.tile([128, 256], fp32, name="p1h")
                    mm_group(p1h, w1h, 256, ents)
                    hh1 = emit_h(p1h, 1280, "hh1")
                    state_update(
                        h1loc[prv], h1loc[cur], zr1[:, 0:256], hh1, first=(t == 1)
                    )
                    h1_sh = work.tile([128, 256], bf16, name="h1_sh")
                    nc.vector.tensor_copy(h1_sh[:], h1loc[cur][:])
                    pend_ag_h1 = ag_shard(h1_sh, "h1")

                if t < t_steps:
                    new_state_tiles(prv)  # rotate buffers for next slot

            # ---------------- final FC ----------------
            h1_final = spool.tile([128, H], bf16, name="h1_final")
            dma_gather_in(h1_final, pend_ag_h1)
            pfc = ph_pool.tile([128, OUT], fp32, name="pfc")
            for kt in range(NHT):
                nc.tensor.matmul(
                    pfc[:],
                    h1_final[:, kt * 128 : (kt + 1) * 128],
                    wfc[:, kt * OUT : (kt + 1) * OUT],
                    start=(kt == 0),
                    stop=(kt == NHT - 1),
                )
            out_sb = work.tile([128, OUT], fp32, name="out_sb")
            nc.vector.tensor_add(out_sb[:], pfc[:], fcbias[:])
            nc.sync.dma_start(d_out[:], out_sb[:])

    nc.compile()
    return nc


_CACHE = {}


def _get_nc(t_steps=T, no_collectives=False):
    key = (t_steps, no_collectives)
    if key not in _CACHE:
        _CACHE[key] = build_kernel(t_steps, no_collectives=no_collectives)
    return _CACHE[key]


def run(inputs, t_steps=T, trace=False):
    from concourse import bass_utils

    nc = _get_nc(t_steps)
    in_maps = host_prep(inputs)
    res = bass_utils.run_bass_kernel_spmd(
        nc, in_maps, core_ids=list(range(NCORES)), trace=trace
    )
    out = np.asarray(res.results[0]["out"], np.float32)
    return out, res


def timed_run(inputs, t_steps=T, iters=4, no_collectives=False, measure_reps=False):
    """Execute via PJRT with a persistent jitted executable; time each call."""
    import time

    import jax
    from jax.sharding import Mesh, PartitionSpec
    from jax.experimental.shard_map import shard_map

    import concourse.mybir as mybir
    from concourse import bass2jax

    nc = _get_nc(t_steps, no_collectives=no_collectives)
    in_maps = host_prep(inputs)
    bass2jax.install_neuronx_cc_hook()

    partition_name = nc.partition_id_tensor.name if nc.partition_id_tensor else None
    in_names, out_names, out_avals, zero_outs = [], [], [], []
    for alloc in nc.m.functions[0].allocations:
        if not isinstance(alloc, mybir.MemoryLocationSet):
            continue
        name = alloc.memorylocations[0].name
        if alloc.kind == "ExternalInput":
            if name != partition_name:
                in_names.append(name)
        elif alloc.kind == "ExternalOutput":
            out_names.append(name)
            shape = tuple(alloc.tensor_shape)
            dtype = mybir.dt.np(alloc.dtype)
            out_avals.append(jax.core.ShapedArray(shape, dtype))
            zero_outs.append(np.zeros(shape, dtype))
    n_params = len(in_names)
    n_outs = len(out_avals)
    all_in_names = list(in_names) + list(out_names)
    if partition_name is not None:
        all_in_names = all_in_names + [partition_name]

    def _body(*args):
        operands = list(args)
        if partition_name is not None:
            operands.append(bass2jax.partition_id_tensor())
        outs = bass2jax._bass_exec_p.bind(
            *operands,
            out_avals=tuple(out_avals),
            in_names=tuple(all_in_names),
            out_names=tuple(out_names),
            lowering_input_output_aliases=(),
            sim_require_finite=True,
            sim_require_nnan=True,
            nc=nc,
        )
        return tuple(outs)

    devices = jax.devices()[:NCORES]
    mesh = Mesh(np.asarray(devices), ("core",))
    in_specs = (PartitionSpec("core"),) * (n_params + n_outs)
    out_specs = (PartitionSpec("core"),) * n_outs
    donate = tuple(range(n_params, n_params + n_outs))
    sharded = jax.jit(
        shard_map(
            _body, mesh=mesh, in_specs=in_specs, out_specs=out_specs, check_rep=False
        ),
        donate_argnums=donate,
        keep_unused=True,
    )
    per_core = [[np.asarray(m[name]) for name in in_names] for m in in_maps]
    concat_in = [
        np.concatenate([per_core[c][i] for c in range(NCORES)], axis=0)
        for i in range(n_params)
    ]
    sharding = jax.sharding.NamedSharding(mesh, PartitionSpec("core"))
    dev_in = [jax.device_put(a, sharding) for a in concat_in]

    def one_call():
        zeros = [
            jax.device_put(
                np.zeros((NCORES * z.shape[0], *z.shape[1:]), z.dtype), sharding
            )
            for z in zero_outs
        ]
        for z in zeros:
            z.block_until_ready()
        t0 = time.perf_counter()
        outs = sharded(*dev_in, *zeros)
        for o in outs:
            o.block_until_ready()
        return time.perf_counter() - t0, outs

    times = []
    outs = None
    for _ in range(iters):
        dt, outs = one_call()
        times.append(dt)
    out0 = np.asarray(outs[0]).reshape(NCORES, *out_avals[0].shape)[0]

    return dict(times=times, best=min(times), out=np.asarray(out0, np.float32))


def kernel(**inputs):
    out, _ = run(inputs)
    return out
